# revision 1
# baseline (speedup 1.0000x reference)
"""Trainium2 Bass kernel for nn_MultiHeadedAttention_51737176047655.

Multi-head attention with Music-Transformer relative position bias
(skew trick), B=4, L=1024, D=1024, 16 heads, head_dim=64.

Sharding (8 cores): core = 2*b + hg  -> batch b in [0,4), head-group hg in
[0,2).  Each core computes 8 heads for one batch over the full sequence:
  - Wq/Wk/Wv column-sharded [1024, 512], Wo row-sharded [512, 1024]
  - per-core output is a partial [1024, 1024]; host sums the two
    head-group partials per batch (standard TP unshard) and adds bo.

Device algorithm per core (matmuls bf16 in / f32 PSUM accumulate):
  qT/kT/vT arrive host-transposed [d, l]; projections give qhT/khT
  [d', l] (transposed) and vh [l, d'] (natural, with a ones column per
  head for softmax sums).  QE = qh e^T is computed per head, tri-masked
  via a shifted-tri "slab" multiply, and written to a DRAM scratch in
  the padded layout (row stride 1025); reading rows back with stride
  1024 materializes the skewed Srel exactly (the reference's
  pad+reshape trick).  scores are computed TRANSPOSED (scores^T =
  kh qh^T, head pairs packed into PE row groups via tile_position) and
  Srel^T is accumulated into the same PSUM bank by transpose-by-identity
  matmuls (lhsT=srel_chunk, rhs=I), skipping statically-zero 128x128
  blocks; exp via ScalarE (scale=1/8) -> unnormalized attn^T (bf16);
  ctx^T_aug = [vh|1]^T attn^T per head (row 64 = softmax denominators
  Z); one DVE reciprocal per head covers both Z rows; 1/Z is broadcast
  across 64 partitions with a step-0-repeat DMA and applied by the DVE
  while packing ctx^T head-pairs; out = ctx Wo.  QE/stripe generation is
  interleaved with the scores pipeline (2-head lead) and attnV lags
  scores by one head so the PE always has independent matmul work.
No max-subtraction in softmax: logits are ~N(0, 1.4^2), far inside
fp32/exp range (validated vs reference at ~1e-6 in fp32 emulation).
"""

import math
import sys

import numpy as np

sys.path.insert(0, "/opt/trn_rl_repo")

import ml_dtypes  # noqa: E402

BF16 = ml_dtypes.bfloat16

# Problem constants (hardcoded per contract)
B = 4
L = 1024
D = 1024
H = 16
HD = 64
H_LOC = 8  # heads per core
DG = 512  # d' columns per core (H_LOC * HD)
NCORES = 8
MAX_SEQ = 2048
PAD = L + 1  # 1025, padded row stride of the skew scratch
FLAT = L * PAD  # 1049600 elements per head scratch

NLT = L // 128  # 8 l-tiles
NDT = D // 128  # 8 contraction tiles
NPAIR = H_LOC // 2  # 4 head pairs


def _build_bass():
    """Build the single-core SPMD Bass program (same program, per-core data)."""
    import concourse.bass as bass
    import concourse.tile as tile
    from concourse import bacc, mybir

    f32 = mybir.dt.float32
    bf16 = mybir.dt.bfloat16
    Exp = mybir.ActivationFunctionType.Exp
    mult = mybir.AluOpType.mult

    nc = bacc.Bacc(
        "TRN2", target_bir_lowering=False, debug=False, enable_asserts=False
    )

    # ---- kernel I/O (qT/kT/vT are host-transposed [d, l]) ----
    qT_d = nc.declare_dram_parameter("qT", [D, L], bf16, isOutput=False)
    kT_d = nc.declare_dram_parameter("kT", [D, L], bf16, isOutput=False)
    vT_d = nc.declare_dram_parameter("vT", [D, L], bf16, isOutput=False)
    wq_d = nc.declare_dram_parameter("wq", [D, DG], bf16, isOutput=False)
    wk_d = nc.declare_dram_parameter("wk", [D, DG], bf16, isOutput=False)
    wv_d = nc.declare_dram_parameter("wv", [D, DG], bf16, isOutput=False)
    wo_d = nc.declare_dram_parameter("wo", [DG, D], bf16, isOutput=False)
    e2_d = nc.declare_dram_parameter("e2", [128, L], bf16, isOutput=False)
    tri_d = nc.declare_dram_parameter("tri", [128, 128], f32, isOutput=False)
    slab_d = nc.declare_dram_parameter("slab", [128, 640], bf16, isOutput=False)
    out_d = nc.declare_dram_parameter("out", [L, D], f32, isOutput=True)

    # skew scratch, one padded buffer per local head
    scratch = [nc.dram_tensor(f"skew{h}", [FLAT], bf16) for h in range(H_LOC)]

    # block (lt, jt) of Srel is identically zero unless piece A
    # (j <= 2l-1023) or piece B (l+2 <= j <= 2l+3) intersects it.
    def srel_block_nonzero(lt, jt):
        l1 = 128 * lt + 127
        j0, j1 = 128 * jt, 128 * jt + 127
        a = 2 * l1 - 1023 >= j0
        b = (j1 >= 128 * lt + 2) and (j0 <= 2 * l1 + 3)
        return a or b

    with tile.TileContext(nc) as tc:
        from contextlib import ExitStack

        with ExitStack() as outer:
            # ---------------- persistent pools ----------------
            persist = outer.enter_context(tc.tile_pool(name="persist", bufs=1))
            # projection outputs (live through whole kernel)
            qhT = persist.tile([128, NPAIR, L], bf16)  # [part, pair, l]
            khT = persist.tile([128, NPAIR, L], bf16)
            # vh with ones column per head: [part(j%128), jt, head, 65]
            vh = persist.tile([128, NLT, H_LOC, HD + 1], bf16)
            e2_sb = persist.tile([128, L], bf16)
            tri_sb = persist.tile([128, 128], f32)
            slab_sb = persist.tile([128, 640], bf16)
            ctxp = persist.tile([128, NPAIR, L], bf16)  # packed ctx^T per pair
            wo_sb = [
                persist.tile([128, D], bf16, name=f"wo{i}") for i in range(NPAIR)
            ]

            nc.sync.dma_start(out=e2_sb, in_=e2_d[:, :])
            nc.sync.dma_start(out=tri_sb, in_=tri_d[:, :])
            nc.sync.dma_start(out=slab_sb, in_=slab_d[:, :])
            for i in range(NPAIR):
                nc.sync.dma_start(out=wo_sb[i], in_=wo_d[128 * i : 128 * (i + 1), :])
            nc.vector.memset(vh[:, :, :, HD : HD + 1], 1.0)

            # ---------------- phase 1+2: loads + projections ----
            with ExitStack() as outer2:
                stp = outer2.enter_context(tc.tile_pool(name="stp", bufs=2))
                sc_ps = outer2.enter_context(
                    tc.tile_pool(name="sc_ps", bufs=6, space="PSUM")
                )
                qe_ps = sc_ps  # QE shares the scores PSUM slots (tag "sc")
                ctx_ps = None  # opened after mm_ps closes (PSUM bank budget)
                attT = outer2.enter_context(tc.tile_pool(name="attT", bufs=4))
                srl = outer2.enter_context(tc.tile_pool(name="srl", bufs=2))
                zp = outer2.enter_context(tc.tile_pool(name="zp", bufs=2))

                ident = persist.tile([128, 128], bf16, name="ident")
                from concourse.masks import make_identity

                make_identity(nc, ident)

                # short-lived input pools opened last (LIFO close order)
                tin_blk = ExitStack()
                tin = tin_blk.enter_context(tc.tile_pool(name="tin", bufs=1))
                mm_ps = tin_blk.enter_context(
                    tc.tile_pool(name="mm_ps", bufs=2, space="PSUM")
                )

                qT = [tin.tile([128, L], bf16, name=f"qT{i}") for i in range(NDT)]
                kT = [tin.tile([128, L], bf16, name=f"kT{i}") for i in range(NDT)]
                vT = [tin.tile([128, L], bf16, name=f"vT{i}") for i in range(NDT)]
                wq_sb = [tin.tile([128, DG], bf16, name=f"wq{i}") for i in range(NDT)]
                wk_sb = [tin.tile([128, DG], bf16, name=f"wk{i}") for i in range(NDT)]
                wv_sb = [tin.tile([128, DG], bf16, name=f"wv{i}") for i in range(NDT)]

                # q + Wq first so qh projections (and QE) can start early
                for i in range(NDT):
                    dsl = slice(128 * i, 128 * (i + 1))
                    nc.sync.dma_start(out=qT[i], in_=qT_d[dsl, :])
                    nc.sync.dma_start(out=wq_sb[i], in_=wq_d[dsl, :])
                for i in range(NDT):
                    dsl = slice(128 * i, 128 * (i + 1))
                    nc.sync.dma_start(out=kT[i], in_=kT_d[dsl, :])
                    nc.sync.dma_start(out=wk_sb[i], in_=wk_d[dsl, :])
                for i in range(NDT):
                    dsl = slice(128 * i, 128 * (i + 1))
                    nc.sync.dma_start(out=vT[i], in_=vT_d[dsl, :])
                    nc.sync.dma_start(out=wv_sb[i], in_=wv_d[dsl, :])

                def proj_pair(w_sb, xT, dst, p):
                    for lh in range(2):
                        ps = mm_ps.tile([128, 512], f32, name="proj_ps", tag="mm")
                        lsl = slice(512 * lh, 512 * (lh + 1))
                        for dt in range(NDT):
                            nc.tensor.matmul(
                                ps,
                                w_sb[dt][:, 128 * p : 128 * (p + 1)],
                                xT[dt][:, lsl],
                                start=(dt == 0),
                                stop=(dt == NDT - 1),
                            )
                        nc.scalar.copy(dst[:, p, lsl], ps)

                def vh_tile(jt):
                    ps = mm_ps.tile([128, 512], f32, name="vh_ps", tag="mm")
                    jsl = slice(128 * jt, 128 * (jt + 1))
                    for dt in range(NDT):
                        nc.tensor.matmul(
                            ps,
                            vT[dt][:, jsl],
                            wv_sb[dt][:, :],
                            start=(dt == 0),
                            stop=(dt == NDT - 1),
                        )
                    # scatter 512 d' columns into per-head [64] slots
                    for h in range(H_LOC):
                        nc.scalar.copy(
                            vh[:, jt, h, 0:HD], ps[:, 64 * h : 64 * (h + 1)]
                        )

                def qe_stripes(h):
                    """QE + masked padded stripes for head h, one batched DMA
                    per 4 l-tiles."""
                    p, hl = divmod(h, 2)
                    rows = slice(64 * hl, 64 * (hl + 1))
                    tp = (64 * hl, 0)
                    for lh in range(2):
                        big = stp.tile([128, 4, PAD], bf16, name="stripe")
                        for a in range(4):
                            lt = 4 * lh + a
                            l0 = 128 * lt
                            lsl = slice(l0, l0 + 128)
                            stripe = big[:, a, :]
                            # QE for the needed m-range only (m < l0+128)
                            nmh = 1 if lt <= 3 else 2
                            ps = [None, None]
                            for mh in range(nmh):
                                psm = qe_ps.tile([128, 512], f32, name="qe", tag="sc")
                                nc.tensor.matmul(
                                    psm,
                                    qhT[rows, p, lsl],
                                    e2_sb[rows, 512 * mh : 512 * (mh + 1)],
                                    start=True,
                                    stop=True,
                                    tile_position=tp,
                                )
                                ps[mh] = psm
                            nc.vector.memset(stripe[:, 0:1], 0.0)
                            # masked QE rows via one shifted-tri slab multiply
                            # per PSUM bank: slab[r, c] = ((c-512) <= r), so
                            # slab[:, 512-l0+m] = (m <= l0+r) = global tri
                            if lt <= 3:
                                nc.vector.tensor_tensor(
                                    stripe[:, 1 : 1 + l0 + 128],
                                    ps[0][:, 0 : l0 + 128],
                                    slab_sb[:, 512 - l0 : 640],
                                    mult,
                                )
                            elif lt == 4:
                                nc.vector.tensor_tensor(
                                    stripe[:, 1:513],
                                    ps[0],
                                    slab_sb[:, 0:512],
                                    mult,
                                )
                                nc.vector.tensor_tensor(
                                    stripe[:, 513 : 1 + l0 + 128],
                                    ps[1][:, 0 : l0 + 128 - 512],
                                    slab_sb[:, 1024 - l0 : 640],
                                    mult,
                                )
                            else:
                                # m < 512 is fully below the diagonal: copy
                                nc.vector.tensor_copy(stripe[:, 1:513], ps[0])
                                nc.vector.tensor_tensor(
                                    stripe[:, 513 : 1 + l0 + 128],
                                    ps[1][:, 0 : l0 + 128 - 512],
                                    slab_sb[:, 1024 - l0 : 640],
                                    mult,
                                )
                            # m > l0+127 : zeros
                            if l0 + 128 < L:
                                nc.vector.memset(stripe[:, 1 + l0 + 128 : PAD], 0.0)
                        # one DMA for the 4 padded stripes
                        dst = bass.AP(
                            tensor=scratch[h],
                            offset=512 * lh * PAD,
                            ap=[[PAD, 128], [128 * PAD, 4], [1, PAD]],
                        )
                        nc.sync.dma_start(out=dst, in_=big)

                def scores_half(h, lh):
                    """scores^T + Srel^T + exp for one l-half of head h."""
                    p, hl = divmod(h, 2)
                    rows = slice(64 * hl, 64 * (hl + 1))
                    tp = (64 * hl, 0)
                    if True:
                        lsl = slice(512 * lh, 512 * (lh + 1))
                        attnT_half = attT.tile([128, NLT, 512], bf16, name="attnT")
                        srel = srl.tile([128, 4, L], bf16, name="srel")
                        if lh == 0:
                            # low l-half: read only the nonzero jt span per lt
                            for a in range(4):
                                lt = a
                                nzj = [jt for jt in range(NLT)
                                       if srel_block_nonzero(lt, jt)]
                                j0, j1 = 128 * min(nzj), 128 * (max(nzj) + 1)
                                src = bass.AP(
                                    tensor=scratch[h],
                                    offset=(128 * lt + 1) * L + j0,
                                    ap=[[L, 128], [1, j1 - j0]],
                                )
                                nc.sync.dma_start(
                                    out=srel[:, a, j0:j1], in_=src
                                )
                        else:
                            # high l-half: dense, one batched DMA
                            src = bass.AP(
                                tensor=scratch[h],
                                offset=(512 * lh + 1) * L,
                                ap=[[L, 128], [128 * L, 4], [1, L]],
                            )
                            nc.sync.dma_start(out=srel, in_=src)
                        for jt in range(NLT):
                            jsl = slice(128 * jt, 128 * (jt + 1))
                            ps = sc_ps.tile([128, 512], f32, name="sc", tag="sc")
                            nzs = [
                                a for a in range(4)
                                if srel_block_nonzero(4 * lh + a, jt)
                            ]
                            # scores^T = kh qh^T for this (j-tile, l-half)
                            nc.tensor.matmul(
                                ps,
                                khT[rows, p, jsl],
                                qhT[rows, p, lsl],
                                start=True,
                                stop=(len(nzs) == 0),
                                tile_position=tp,
                            )
                            # += Srel^T via PE transpose-by-identity
                            for i, a in enumerate(nzs):
                                nc.tensor.matmul(
                                    ps[:, 128 * a : 128 * a + 128],
                                    srel[:, a, jsl],
                                    ident,
                                    start=False,
                                    stop=(i == len(nzs) - 1),
                                )
                            nc.scalar.activation(
                                attnT_half[:, jt, :], ps, Exp, scale=0.125
                            )
                    return attnT_half

                def attnv_head(h, halves):
                    p, hl = divmod(h, 2)
                    rows = slice(64 * hl, 64 * (hl + 1))
                    cps_all = {}
                    zstack = zp.tile([64, 512], f32, name="zstack")
                    nc.vector.memset(zstack, 1.0)
                    for lh in range(2):
                        cps = ctx_ps.tile([128, 512], f32, name="cps", tag="cps")
                        for jt in range(NLT):
                            nc.tensor.matmul(
                                cps[0 : HD + 1, :],
                                vh[:, jt, h, :],
                                halves[lh][:, jt, :],
                                start=(jt == 0),
                                stop=(jt == NLT - 1),
                            )
                        nc.scalar.copy(
                            zstack[32 * lh : 32 * lh + 1, :], cps[HD : HD + 1, :]
                        )
                        cps_all[lh] = cps
                    # one reciprocal covers both Z rows (partitions 0 and 32)
                    zinv = zp.tile([64, 512], f32, name="zinv")
                    nc.vector.reciprocal(zinv, zstack)
                    for lh in range(2):
                        zbc = zp.tile([64, 512], f32, name="zbc")
                        # broadcast across 64 partitions via a
                        # step-0 repeat dim (legal for DMA only)
                        zr = zinv[32 * lh : 32 * lh + 1, :]
                        zrow_bc = bass.AP(
                            tensor=zr.tensor,
                            offset=zr.offset,
                            ap=[list(zr.ap)[0], [0, 64]] + list(zr.ap)[1:],
                        )
                        nc.sync.dma_start(out=zbc, in_=zrow_bc)
                        # normalize + pack into head-pair ctx^T (bf16)
                        nc.vector.tensor_tensor(
                            ctxp[rows, p, 512 * lh : 512 * (lh + 1)],
                            cps_all[lh][0:HD, :],
                            zbc,
                            mult,
                        )

                # ---- emission: projections, then a merged stripes/scores/
                # attnV pipeline so PE always has dense independent work ----
                for p in range(NPAIR):
                    proj_pair(wq_sb, qT, qhT, p)
                qe_stripes(0)
                qe_stripes(1)
                for p in range(NPAIR):
                    proj_pair(wk_sb, kT, khT, p)
                for jt in range(NLT):
                    vh_tile(jt)
                tin_blk.close()
                ctx_ps = outer2.enter_context(
                    tc.tile_pool(name="ctx_ps", bufs=2, space="PSUM")
                )

                pend = None
                for h in range(H_LOC):
                    a0 = scores_half(h, 0)
                    a1 = scores_half(h, 1)
                    if h + 2 < H_LOC:
                        qe_stripes(h + 2)
                    if pend is not None:
                        attnv_head(*pend)
                    pend = (h, [a0, a1])
                attnv_head(*pend)

            # ---------------- phase 6: output projection --------------------
            with ExitStack() as ph:
                op_ps = ph.enter_context(
                    tc.tile_pool(name="op_ps", bufs=8, space="PSUM")
                )
                ost = ph.enter_context(tc.tile_pool(name="ost", bufs=4))

                for lt in range(NLT):
                    lsl = slice(128 * lt, 128 * (lt + 1))
                    for jh in range(2):
                        jsl = slice(512 * jh, 512 * (jh + 1))
                        ps = op_ps.tile([128, 512], f32, name="op", tag="op")
                        for p in range(NPAIR):
                            nc.tensor.matmul(
                                ps,
                                ctxp[:, p, lsl],
                                wo_sb[p][:, jsl],
                                start=(p == 0),
                                stop=(p == NPAIR - 1),
                            )
                        o = ost.tile([128, 512], f32, name="o")
                        nc.scalar.copy(o, ps)
                        nc.sync.dma_start(out=out_d[lsl, jsl], in_=o)

    nc.compile()
    return nc


TRACE = False
TRACE_KWARGS = {}
LAST_RESULT = None

_NC_CACHE = None


def _get_nc():
    global _NC_CACHE
    if _NC_CACHE is None:
        _NC_CACHE = _build_bass()
    return _NC_CACHE


def make_in_maps(k, v, q, E, Wk, Wv, Wq, Wo):
    """Host-side sharding: returns per-core input dicts."""
    eT = np.ascontiguousarray(E[MAX_SEQ - L :, :].T)  # [64, 1024]
    e2 = np.concatenate([eT, eT], axis=0).astype(BF16)  # [128, 1024]
    tri = (np.arange(128)[None, :] <= np.arange(128)[:, None]).astype(np.float32)
    slab = (
        (np.arange(640)[None, :] - 512) <= np.arange(128)[:, None]
    ).astype(BF16)
    qkvT = {}
    for b in range(B):
        qkvT[b] = (
            np.ascontiguousarray(np.asarray(q[b]).T).astype(BF16),
            np.ascontiguousarray(np.asarray(k[b]).T).astype(BF16),
            np.ascontiguousarray(np.asarray(v[b]).T).astype(BF16),
        )
    in_maps = []
    for core in range(NCORES):
        b, hg = divmod(core, 2)
        csl = slice(DG * hg, DG * (hg + 1))
        qTb, kTb, vTb = qkvT[b]
        in_maps.append(
            {
                "qT": qTb,
                "kT": kTb,
                "vT": vTb,
                "wq": np.ascontiguousarray(Wq[:, csl]).astype(BF16),
                "wk": np.ascontiguousarray(Wk[:, csl]).astype(BF16),
                "wv": np.ascontiguousarray(Wv[:, csl]).astype(BF16),
                "wo": np.ascontiguousarray(Wo[DG * hg : DG * (hg + 1), :]).astype(BF16),
                "e2": e2,
                "tri": tri,
                "slab": slab,
            }
        )
    return in_maps


def kernel(
    k,
    v,
    q,
    mask,
    E,
    Wk,
    bk,
    Wv,
    bv,
    Wq,
    bq,
    Wo,
    bo,
):
    k = np.asarray(k, np.float32)
    v = np.asarray(v, np.float32)
    q = np.asarray(q, np.float32)
    E = np.asarray(E, np.float32)
    Wk = np.asarray(Wk, np.float32)
    Wv = np.asarray(Wv, np.float32)
    Wq = np.asarray(Wq, np.float32)
    Wo = np.asarray(Wo, np.float32)
    mask = np.asarray(mask)
    assert bool(mask.all()), "kernel specialized for all-true mask"
    for bias in (bk, bv, bq):
        assert not np.any(np.asarray(bias)), "kernel specialized for zero qkv biases"
    bo = np.asarray(bo, np.float32)

    from concourse.bass_utils import run_bass_kernel_spmd

    nc = _get_nc()
    in_maps = make_in_maps(k, v, q, E, Wk, Wv, Wq, Wo)
    res = run_bass_kernel_spmd(
        nc, in_maps, core_ids=list(range(NCORES)), trace=TRACE, **TRACE_KWARGS
    )
    global LAST_RESULT
    LAST_RESULT = res
    out = np.zeros((B, L, D), np.float32)
    for core in range(NCORES):
        b = core // 2
        out[b] += res.results[core]["out"]
    out += bo[None, None, :]
    return out



# revision 17
# speedup vs baseline: 1.0411x; 1.0411x over previous
"""Trainium2 Bass kernel for nn_MultiHeadedAttention_51737176047655.

Multi-head attention with Music-Transformer relative position bias
(skew trick), B=4, L=1024, D=1024, 16 heads, head_dim=64.

Sharding (8 cores): core = 2*b + hg  -> batch b in [0,4), head-group hg in
[0,2).  Each core computes 8 heads for one batch over the full sequence:
  - Wq/Wk/Wv column-sharded [1024, 512], Wo row-sharded [512, 1024]
  - per-core output is a partial [1024, 1024]; host sums the two
    head-group partials per batch (standard TP unshard) and adds bo.

Device algorithm per core (matmuls bf16 in / f32 PSUM accumulate):
  qT/kT/vT arrive host-transposed [d, l]; projections give qhT/khT
  [d', l] (transposed) and vh [l, d'] (natural, with a ones column per
  head for softmax sums).  QE = qh e^T is computed per head (only the
  m < l0+128 span that survives the tri mask), masked via a shifted-tri
  "slab" multiply, and written to a DRAM scratch in the padded layout
  (row stride 1025); reading rows back with stride 1024 materializes
  the skewed Srel exactly (the reference's pad+reshape trick).  scores
  are computed TRANSPOSED (scores^T = kh qh^T, head pairs packed into
  PE row groups via tile_position) and Srel^T is accumulated into the
  same PSUM bank by transpose-by-identity matmuls, skipping
  statically-zero 128x128 blocks; exp via ScalarE (scale=1/8) ->
  unnormalized attn^T (bf16); ctx^T_aug = [vh|1]^T attn^T per head
  (row 64 = softmax denominators Z); 1/Z via one small [2,512] DVE
  reciprocal per head, broadcast across 64 partitions with a K=2 PE
  matmul into PSUM, and applied by the DVE while packing ctx^T
  head-pairs; out = ctx Wo, emitted bf16 (host accumulates in f32).
  DMA count is minimized: batched input loads, one sliding-window
  batched read for the low-half Srel stripes, persistent pre-zeroed
  stripe staging tiles.  QE/stripe generation is interleaved with the
  scores pipeline (2-head lead) and attnV lags scores by one head so
  the PE always has independent matmul work.
No max-subtraction in softmax: logits are ~N(0, 1.4^2), far inside
fp32/exp range (validated vs reference at ~1e-6 in fp32 emulation).
"""

import math
import sys

import numpy as np

sys.path.insert(0, "/opt/trn_rl_repo")

import ml_dtypes  # noqa: E402

BF16 = ml_dtypes.bfloat16

# Problem constants (hardcoded per contract)
B = 4
L = 1024
D = 1024
H = 16
HD = 64
H_LOC = 8  # heads per core
DG = 512  # d' columns per core (H_LOC * HD)
NCORES = 8
MAX_SEQ = 2048
PAD = L + 1  # 1025, padded row stride of the skew scratch
FLAT = L * PAD  # 1049600 elements per head scratch

NLT = L // 128  # 8 l-tiles
NDT = D // 128  # 8 contraction tiles
NPAIR = H_LOC // 2  # 4 head pairs


def _build_bass():
    """Build the single-core SPMD Bass program (same program, per-core data)."""
    import concourse.bass as bass
    import concourse.tile as tile
    from concourse import bacc, mybir

    f32 = mybir.dt.float32
    bf16 = mybir.dt.bfloat16
    Exp = mybir.ActivationFunctionType.Exp
    mult = mybir.AluOpType.mult

    nc = bacc.Bacc(
        "TRN2", target_bir_lowering=False, debug=False, enable_asserts=False
    )

    # ---- kernel I/O (qT/kT/vT are host-transposed [d, l]) ----
    qT_d = nc.declare_dram_parameter("qT", [D, L], bf16, isOutput=False)
    kT_d = nc.declare_dram_parameter("kT", [D, L], bf16, isOutput=False)
    vT_d = nc.declare_dram_parameter("vT", [D, L], bf16, isOutput=False)
    wq_d = nc.declare_dram_parameter("wq", [D, DG], bf16, isOutput=False)
    wk_d = nc.declare_dram_parameter("wk", [D, DG], bf16, isOutput=False)
    wv_d = nc.declare_dram_parameter("wv", [D, DG], bf16, isOutput=False)
    wo_d = nc.declare_dram_parameter("wo", [DG, D], bf16, isOutput=False)
    e2_d = nc.declare_dram_parameter("e2", [128, L], bf16, isOutput=False)
    slab_d = nc.declare_dram_parameter("slab", [128, 640], bf16, isOutput=False)
    out_d = nc.declare_dram_parameter("out", [L, D], bf16, isOutput=True)

    # skew scratch, one padded buffer per local head
    scratch = [nc.dram_tensor(f"skew{h}", [FLAT], bf16) for h in range(H_LOC)]

    # block (lt, jt) of Srel is identically zero unless piece A
    # (j <= 2l-1023) or piece B (l+2 <= j <= 2l+3) intersects it.
    def srel_block_nonzero(lt, jt):
        l1 = 128 * lt + 127
        j0, j1 = 128 * jt, 128 * jt + 127
        a = 2 * l1 - 1023 >= j0
        b = (j1 >= 128 * lt + 2) and (j0 <= 2 * l1 + 3)
        return a or b

    with tile.TileContext(nc) as tc:
        from contextlib import ExitStack

        with ExitStack() as outer:
            # ---------------- persistent pools ----------------
            persist = outer.enter_context(tc.tile_pool(name="persist", bufs=1))
            # projection outputs (live through whole kernel)
            qhT = persist.tile([128, NPAIR, L], bf16)  # [part, pair, l]
            khT = persist.tile([128, NPAIR, L], bf16)
            # vh with ones column per head: [part(j%128), jt, head, 65]
            vh = persist.tile([128, NLT, H_LOC, HD + 1], bf16)
            e2_sb = persist.tile([128, L], bf16)
            slab_sb = persist.tile([128, 640], bf16)
            ctxp = persist.tile([128, NPAIR, L], bf16)  # packed ctx^T per pair
            # all-ones stationary for the K=1 1/Z PE broadcast
            ones1 = persist.tile([1, 64], bf16, name="ones1")
            # persistent stripe staging tiles, pre-zeroed once; data spans are
            # rewritten per head, zero-col/tail spans stay zero across reuse
            stripes = [
                [persist.tile([128, 4, PAD], bf16, name=f"st{lh}{par}")
                 for par in range(2)]
                for lh in range(2)
            ]

            nc.sync.dma_start(out=e2_sb, in_=e2_d[:, :])
            nc.sync.dma_start(out=slab_sb, in_=slab_d[:, :])
            nc.vector.memset(vh[:, :, :, HD : HD + 1], 1.0)
            nc.vector.memset(ones1, 1.0)
            for lh in range(2):
                for par in range(2):
                    nc.vector.memset(stripes[lh][par], 0.0)

            # ---------------- phase 1+2: loads + projections ----
            with ExitStack() as outer2:
                sc_ps = outer2.enter_context(
                    tc.tile_pool(name="sc_ps", bufs=5, space="PSUM")
                )
                qe_ps = sc_ps  # QE shares the scores PSUM slots (tag "sc")
                ctx_ps = None  # opened after mm_ps closes (PSUM bank budget)
                zbc_ps = None
                attT = outer2.enter_context(tc.tile_pool(name="attT", bufs=4))
                srl = outer2.enter_context(tc.tile_pool(name="srl", bufs=2))
                zp = outer2.enter_context(tc.tile_pool(name="zp", bufs=2))

                ident = persist.tile([128, 128], bf16, name="ident")
                from concourse.masks import make_identity

                make_identity(nc, ident)

                # short-lived input pools opened last (LIFO close order)
                tin_blk = ExitStack()
                tin = tin_blk.enter_context(tc.tile_pool(name="tin", bufs=1))
                mm_ps = tin_blk.enter_context(
                    tc.tile_pool(name="mm_ps", bufs=2, space="PSUM")
                )

                qT = tin.tile([128, NDT, L], bf16, name="qT")
                kT = tin.tile([128, NDT, L], bf16, name="kT")
                vT = tin.tile([128, NDT, L], bf16, name="vT")
                wq_sb = tin.tile([128, NDT, DG], bf16, name="wq")
                wk_sb = tin.tile([128, NDT, DG], bf16, name="wk")
                wv_sb = tin.tile([128, NDT, DG], bf16, name="wv")

                def load_xT(dst, src_d, n0, n1):
                    src = bass.AP(
                        tensor=src_d,
                        offset=n0 * 128 * L,
                        ap=[[L, 128], [128 * L, n1 - n0], [1, L]],
                    )
                    nc.sync.dma_start(out=dst[:, n0:n1, :], in_=src)

                def load_w(dst, src_d):
                    src = bass.AP(
                        tensor=src_d,
                        offset=0,
                        ap=[[DG, 128], [128 * DG, NDT], [1, DG]],
                    )
                    nc.sync.dma_start(out=dst, in_=src)

                # q + Wq first so qh projections (and QE) can start early;
                # batched loads (one or two triggers per tensor)
                load_w(wq_sb, wq_d)
                load_xT(qT, qT_d, 0, 4)
                load_xT(qT, qT_d, 4, 8)
                load_w(wk_sb, wk_d)
                load_xT(kT, kT_d, 0, 4)
                load_xT(kT, kT_d, 4, 8)
                load_w(wv_sb, wv_d)
                load_xT(vT, vT_d, 0, 8)

                def proj_pair(w_sb, xT, dst, p):
                    for lh in range(2):
                        ps = mm_ps.tile([128, 512], f32, name="proj_ps", tag="mm")
                        lsl = slice(512 * lh, 512 * (lh + 1))
                        for dt in range(NDT):
                            nc.tensor.matmul(
                                ps,
                                w_sb[:, dt, 128 * p : 128 * (p + 1)],
                                xT[:, dt, lsl],
                                start=(dt == 0),
                                stop=(dt == NDT - 1),
                            )
                        nc.vector.tensor_copy(dst[:, p, lsl], ps)

                def vh_tile(jt):
                    ps = mm_ps.tile([128, 512], f32, name="vh_ps", tag="mm")
                    jsl = slice(128 * jt, 128 * (jt + 1))
                    for dt in range(NDT):
                        nc.tensor.matmul(
                            ps,
                            vT[:, dt, jsl],
                            wv_sb[:, dt, :],
                            start=(dt == 0),
                            stop=(dt == NDT - 1),
                        )
                    # scatter 512 d' columns into per-head [64] slots with one
                    # strided copy (dst strides over the 65-wide head slots)
                    pv = ps[0:128, :]
                    ps3 = bass.AP(
                        tensor=pv.tensor,
                        offset=pv.offset,
                        ap=[list(pv.ap)[0], [HD, H_LOC], [1, HD]],
                    )
                    nc.scalar.copy(vh[:, jt, :, 0:HD], ps3)

                def qe_stripes(h):
                    """QE + masked padded stripes for head h, one batched DMA
                    per 4 l-tiles."""
                    p, hl = divmod(h, 2)
                    rows = slice(64 * hl, 64 * (hl + 1))
                    tp = (64 * hl, 0)
                    for lh in range(2):
                        big = stripes[lh][h % 2]
                        for a in range(4):
                            lt = 4 * lh + a
                            l0 = 128 * lt
                            lsl = slice(l0, l0 + 128)
                            stripe = big[:, a, :]
                            # QE only over the m-range that survives the tri
                            # mask for this row block (m < l0+128)
                            ps = [None, None]
                            if lt <= 3:
                                n0 = l0 + 128
                                psm = qe_ps.tile([128, 512], f32, name="qe", tag="sc")
                                nc.tensor.matmul(
                                    psm[:, 0:n0],
                                    qhT[rows, p, lsl],
                                    e2_sb[rows, 0:n0],
                                    start=True,
                                    stop=True,
                                    tile_position=tp,
                                )
                                ps[0] = psm
                            else:
                                psm = qe_ps.tile([128, 512], f32, name="qe", tag="sc")
                                nc.tensor.matmul(
                                    psm,
                                    qhT[rows, p, lsl],
                                    e2_sb[rows, 0:512],
                                    start=True,
                                    stop=True,
                                    tile_position=tp,
                                )
                                ps[0] = psm
                                n1 = l0 + 128 - 512
                                psm = qe_ps.tile([128, 512], f32, name="qe", tag="sc")
                                nc.tensor.matmul(
                                    psm[:, 0:n1],
                                    qhT[rows, p, lsl],
                                    e2_sb[rows, 512 : 512 + n1],
                                    start=True,
                                    stop=True,
                                    tile_position=tp,
                                )
                                ps[1] = psm
                            # masked QE rows via one shifted-tri slab multiply
                            # per PSUM bank: slab[r, c] = ((c-512) <= r), so
                            # slab[:, 512-l0+m] = (m <= l0+r) = global tri
                            if lt <= 3:
                                nc.vector.tensor_tensor(
                                    stripe[:, 1 : 1 + l0 + 128],
                                    ps[0][:, 0 : l0 + 128],
                                    slab_sb[:, 512 - l0 : 640],
                                    mult,
                                )
                            elif lt == 4:
                                nc.vector.tensor_tensor(
                                    stripe[:, 1:513],
                                    ps[0],
                                    slab_sb[:, 0:512],
                                    mult,
                                )
                                nc.vector.tensor_tensor(
                                    stripe[:, 513 : 1 + l0 + 128],
                                    ps[1][:, 0 : l0 + 128 - 512],
                                    slab_sb[:, 1024 - l0 : 640],
                                    mult,
                                )
                            else:
                                # m < 512 is fully below the diagonal: copy
                                nc.vector.tensor_copy(stripe[:, 1:513], ps[0])
                                nc.vector.tensor_tensor(
                                    stripe[:, 513 : 1 + l0 + 128],
                                    ps[1][:, 0 : l0 + 128 - 512],
                                    slab_sb[:, 1024 - l0 : 640],
                                    mult,
                                )
                        # one DMA for the 4 padded stripes
                        dst = bass.AP(
                            tensor=scratch[h],
                            offset=512 * lh * PAD,
                            ap=[[PAD, 128], [128 * PAD, 4], [1, PAD]],
                        )
                        nc.sync.dma_start(out=dst, in_=big)

                def scores_half(h, lh):
                    """scores^T + Srel^T + exp for one l-half of head h."""
                    p, hl = divmod(h, 2)
                    rows = slice(64 * hl, 64 * (hl + 1))
                    tp = (64 * hl, 0)
                    lsl = slice(512 * lh, 512 * (lh + 1))
                    attnT_half = attT.tile([128, NLT, 512], bf16, name="attnT")
                    if lh == 0:
                        # low l-half: sliding 640-wide j-window per lt
                        # (window start 128*lt covers all nonzero blocks),
                        # one batched DMA
                        srel = srl.tile([128, 4, 640], bf16, name="srel")
                        src = bass.AP(
                            tensor=scratch[h],
                            offset=L,
                            ap=[[L, 128], [128 * L + 128, 4], [1, 640]],
                        )
                        nc.sync.dma_start(out=srel, in_=src)
                    else:
                        # high l-half: dense, one batched DMA
                        srel = srl.tile([128, 4, L], bf16, name="srel")
                        src = bass.AP(
                            tensor=scratch[h],
                            offset=(512 * lh + 1) * L,
                            ap=[[L, 128], [128 * L, 4], [1, L]],
                        )
                        nc.sync.dma_start(out=srel, in_=src)
                    for jt in range(NLT):
                        jsl = slice(128 * jt, 128 * (jt + 1))
                        ps = sc_ps.tile([128, 512], f32, name="sc", tag="sc")
                        nzs = [
                            a for a in range(4)
                            if srel_block_nonzero(4 * lh + a, jt)
                        ]
                        # scores^T = kh qh^T for this (j-tile, l-half)
                        nc.tensor.matmul(
                            ps,
                            khT[rows, p, jsl],
                            qhT[rows, p, lsl],
                            start=True,
                            stop=(len(nzs) == 0),
                            tile_position=tp,
                        )
                        # += Srel^T via PE transpose-by-identity
                        for i, a in enumerate(nzs):
                            if lh == 0:
                                # window-relative j columns
                                jr = slice(128 * (jt - a), 128 * (jt - a) + 128)
                                chunk = srel[:, a, jr]
                            else:
                                chunk = srel[:, a, jsl]
                            nc.tensor.matmul(
                                ps[:, 128 * a : 128 * a + 128],
                                chunk,
                                ident,
                                start=False,
                                stop=(i == len(nzs) - 1),
                            )
                        nc.scalar.activation(
                            attnT_half[:, jt, :], ps, Exp, scale=0.125
                        )
                    return attnT_half

                def attnv_head(h, halves):
                    p, hl = divmod(h, 2)
                    rows = slice(64 * hl, 64 * (hl + 1))
                    cps_all = {}
                    zinvb_all = {}
                    for lh in range(2):
                        cps = ctx_ps.tile([128, 512], f32, name="cps", tag="cps")
                        for jt in range(NLT):
                            nc.tensor.matmul(
                                cps[0 : HD + 1, :],
                                vh[:, jt, h, :],
                                halves[lh][:, jt, :],
                                start=(jt == 0),
                                stop=(jt == NLT - 1),
                            )
                        # small per-half reciprocal of the Z row
                        zs = zp.tile([1, 512], f32, name="zs")
                        nc.scalar.copy(zs, cps[HD : HD + 1, :])
                        zinv = zp.tile([1, 512], f32, name="zinv")
                        nc.vector.reciprocal(zinv, zs)
                        zinvb = zp.tile([1, 512], bf16, name="zinvb")
                        nc.vector.tensor_copy(zinvb, zinv)
                        cps_all[lh] = cps
                        zinvb_all[lh] = zinvb
                    for lh in range(2):
                        # broadcast 1/Z across 64 partitions with a K=1 matmul
                        zb = zbc_ps.tile([64, 512], f32, name="zb", tag="zb")
                        nc.tensor.matmul(
                            zb,
                            ones1,
                            zinvb_all[lh],
                            start=True,
                            stop=True,
                        )
                        zbs = zp.tile([64, 512], bf16, name="zbs")
                        nc.scalar.copy(zbs, zb)
                        # normalize + pack into head-pair ctx^T (bf16)
                        nc.vector.tensor_tensor(
                            ctxp[rows, p, 512 * lh : 512 * (lh + 1)],
                            cps_all[lh][0:HD, :],
                            zbs,
                            mult,
                        )

                # ---- emission: projections, then a merged stripes/scores/
                # attnV pipeline so PE always has dense independent work ----
                for p in range(NPAIR):
                    proj_pair(wq_sb, qT, qhT, p)
                qe_stripes(0)
                qe_stripes(1)
                for p in range(NPAIR):
                    proj_pair(wk_sb, kT, khT, p)
                for jt in range(NLT):
                    vh_tile(jt)
                tin_blk.close()
                ctx_ps = outer2.enter_context(
                    tc.tile_pool(name="ctx_ps", bufs=2, space="PSUM")
                )
                zbc_ps = outer2.enter_context(
                    tc.tile_pool(name="zbc_ps", bufs=1, space="PSUM")
                )
                # wo lives in the space vacated by the input tiles; loaded
                # here (well before phase 6)
                wop = outer2.enter_context(tc.tile_pool(name="wop", bufs=1))
                wo_sb = wop.tile([128, NPAIR, D], bf16, name="wo")
                wo_src = bass.AP(
                    tensor=wo_d, offset=0, ap=[[D, 128], [128 * D, NPAIR], [1, D]]
                )
                nc.sync.dma_start(out=wo_sb, in_=wo_src)

                ost = outer2.enter_context(tc.tile_pool(name="ost", bufs=4))

                pend = None
                for h in range(H_LOC):
                    a0 = scores_half(h, 0)
                    a1 = scores_half(h, 1)
                    if h + 2 < H_LOC:
                        qe_stripes(h + 2)
                    if pend is not None:
                        attnv_head(*pend)
                    pend = (h, [a0, a1])
                attnv_head(*pend)

                # ------------ phase 6: output projection (reuses sc PSUM) ----
                for lt in range(NLT):
                    lsl = slice(128 * lt, 128 * (lt + 1))
                    o = ost.tile([128, D], bf16, name="o")
                    for jh in range(2):
                        jsl = slice(512 * jh, 512 * (jh + 1))
                        ps = sc_ps.tile([128, 512], f32, name="op", tag="sc")
                        for p in range(NPAIR):
                            nc.tensor.matmul(
                                ps,
                                ctxp[:, p, lsl],
                                wo_sb[:, p, jsl],
                                start=(p == 0),
                                stop=(p == NPAIR - 1),
                            )
                        nc.vector.tensor_copy(o[:, jsl], ps)
                    nc.sync.dma_start(out=out_d[lsl, :], in_=o)

    nc.compile()
    return nc


TRACE = False
TRACE_KWARGS = {}
LAST_RESULT = None

_NC_CACHE = None


def _get_nc():
    global _NC_CACHE
    if _NC_CACHE is None:
        _NC_CACHE = _build_bass()
    return _NC_CACHE


def make_in_maps(k, v, q, E, Wk, Wv, Wq, Wo):
    """Host-side sharding: returns per-core input dicts."""
    eT = np.ascontiguousarray(E[MAX_SEQ - L :, :].T)  # [64, 1024]
    e2 = np.concatenate([eT, eT], axis=0).astype(BF16)  # [128, 1024]
    slab = (
        (np.arange(640)[None, :] - 512) <= np.arange(128)[:, None]
    ).astype(BF16)
    qkvT = {}
    for b in range(B):
        qkvT[b] = (
            np.ascontiguousarray(np.asarray(q[b]).T).astype(BF16),
            np.ascontiguousarray(np.asarray(k[b]).T).astype(BF16),
            np.ascontiguousarray(np.asarray(v[b]).T).astype(BF16),
        )
    in_maps = []
    for core in range(NCORES):
        b, hg = divmod(core, 2)
        csl = slice(DG * hg, DG * (hg + 1))
        qTb, kTb, vTb = qkvT[b]
        in_maps.append(
            {
                "qT": qTb,
                "kT": kTb,
                "vT": vTb,
                "wq": np.ascontiguousarray(Wq[:, csl]).astype(BF16),
                "wk": np.ascontiguousarray(Wk[:, csl]).astype(BF16),
                "wv": np.ascontiguousarray(Wv[:, csl]).astype(BF16),
                "wo": np.ascontiguousarray(Wo[DG * hg : DG * (hg + 1), :]).astype(BF16),
                "e2": e2,
                "slab": slab,
            }
        )
    return in_maps


def kernel(
    k,
    v,
    q,
    mask,
    E,
    Wk,
    bk,
    Wv,
    bv,
    Wq,
    bq,
    Wo,
    bo,
):
    k = np.asarray(k, np.float32)
    v = np.asarray(v, np.float32)
    q = np.asarray(q, np.float32)
    E = np.asarray(E, np.float32)
    Wk = np.asarray(Wk, np.float32)
    Wv = np.asarray(Wv, np.float32)
    Wq = np.asarray(Wq, np.float32)
    Wo = np.asarray(Wo, np.float32)
    mask = np.asarray(mask)
    assert bool(mask.all()), "kernel specialized for all-true mask"
    for bias in (bk, bv, bq):
        assert not np.any(np.asarray(bias)), "kernel specialized for zero qkv biases"
    bo = np.asarray(bo, np.float32)

    from concourse.bass_utils import run_bass_kernel_spmd

    nc = _get_nc()
    in_maps = make_in_maps(k, v, q, E, Wk, Wv, Wq, Wo)
    res = run_bass_kernel_spmd(
        nc, in_maps, core_ids=list(range(NCORES)), trace=TRACE, **TRACE_KWARGS
    )
    global LAST_RESULT
    LAST_RESULT = res
    out = np.zeros((B, L, D), np.float32)
    for core in range(NCORES):
        b = core // 2
        out[b] += np.asarray(res.results[core]["out"], np.float32)
    out += bo[None, None, :]
    return out


# revision 20
# speedup vs baseline: 1.0474x; 1.0060x over previous
"""Trainium2 Bass kernel for nn_MultiHeadedAttention_51737176047655.

Multi-head attention with Music-Transformer relative position bias
(skew trick), B=4, L=1024, D=1024, 16 heads, head_dim=64.

Sharding (8 cores): core = 2*b + hg  -> batch b in [0,4), head-group hg in
[0,2).  Each core computes 8 heads for one batch over the full sequence:
  - Wq/Wk/Wv column-sharded [1024, 512], Wo row-sharded [512, 1024]
  - per-core output is a partial [1024, 1024]; host sums the two
    head-group partials per batch (standard TP unshard) and adds bo.

Device algorithm per core (matmuls bf16 in / f32 PSUM accumulate):
  qT/kT/vT arrive host-transposed [d, l]; projections give qhT/khT
  [d', l] (transposed) and vh [l, d'] (natural, with a ones column per
  head for softmax sums).  QE = qh e^T is computed per head (only the
  m < l0+128 span that survives the tri mask), masked via a shifted-tri
  "slab" multiply, and written to a DRAM scratch in the padded layout
  (row stride 1025); reading rows back with stride 1024 materializes
  the skewed Srel exactly (the reference's pad+reshape trick).  scores
  are computed TRANSPOSED (scores^T = kh qh^T, head pairs packed into
  PE row groups via tile_position) and Srel^T is accumulated into the
  same PSUM bank by transpose-by-identity matmuls, skipping
  statically-zero 128x128 blocks; exp via ScalarE (scale=1/8) ->
  unnormalized attn^T (bf16); ctx^T_aug = [vh|1]^T attn^T per head
  (row 64 = softmax denominators Z); 1/Z via a single ScalarE
  Reciprocal activation straight off the PSUM Z row, broadcast across
  64 partitions with a K=1 PE matmul, and applied by the DVE while
  packing ctx^T head-pairs; out = ctx Wo, emitted bf16 (host
  accumulates in f32).
  The TensorE instruction stream is interleaved at j-tile granularity
  (scores of head h / attnV of head h-1 / QE of head h+2, and attnV of
  the last head with the first half of the output projection) so the
  in-order PE never stalls on the exp/normalize pipelines; this keeps
  the PE clock at its top p-state.  DMA count is minimized: batched
  input loads, one sliding-window batched read for the low-half Srel
  stripes, persistent pre-zeroed stripe staging tiles.
No max-subtraction in softmax: logits are ~N(0, 1.4^2), far inside
fp32/exp range (validated vs reference at ~1e-6 in fp32 emulation).
"""

import math
import sys

import numpy as np

sys.path.insert(0, "/opt/trn_rl_repo")

import ml_dtypes  # noqa: E402

BF16 = ml_dtypes.bfloat16

# Problem constants (hardcoded per contract)
B = 4
L = 1024
D = 1024
H = 16
HD = 64
H_LOC = 8  # heads per core
DG = 512  # d' columns per core (H_LOC * HD)
NCORES = 8
MAX_SEQ = 2048
PAD = L + 1  # 1025, padded row stride of the skew scratch
FLAT = L * PAD  # 1049600 elements per head scratch

NLT = L // 128  # 8 l-tiles
NDT = D // 128  # 8 contraction tiles
NPAIR = H_LOC // 2  # 4 head pairs


def _build_bass():
    """Build the single-core SPMD Bass program (same program, per-core data)."""
    import concourse.bass as bass
    import concourse.tile as tile
    from concourse import bacc, mybir

    f32 = mybir.dt.float32
    bf16 = mybir.dt.bfloat16
    Exp = mybir.ActivationFunctionType.Exp
    Ln = mybir.ActivationFunctionType.Ln
    mult = mybir.AluOpType.mult

    nc = bacc.Bacc(
        "TRN2", target_bir_lowering=False, debug=False, enable_asserts=False
    )

    # ---- kernel I/O (qT/kT/vT are host-transposed [d, l]) ----
    qT_d = nc.declare_dram_parameter("qT", [D, L], bf16, isOutput=False)
    kT_d = nc.declare_dram_parameter("kT", [D, L], bf16, isOutput=False)
    vT_d = nc.declare_dram_parameter("vT", [D, L], bf16, isOutput=False)
    wq_d = nc.declare_dram_parameter("wq", [D, DG], bf16, isOutput=False)
    wk_d = nc.declare_dram_parameter("wk", [D, DG], bf16, isOutput=False)
    wv_d = nc.declare_dram_parameter("wv", [D, DG], bf16, isOutput=False)
    wo_d = nc.declare_dram_parameter("wo", [DG, D], bf16, isOutput=False)
    e2_d = nc.declare_dram_parameter("e2", [128, L], bf16, isOutput=False)
    slab_d = nc.declare_dram_parameter("slab", [128, 640], bf16, isOutput=False)
    out_d = nc.declare_dram_parameter("out", [L, D], bf16, isOutput=True)

    # skew scratch, one padded buffer per local head
    scratch = [nc.dram_tensor(f"skew{h}", [FLAT], bf16) for h in range(H_LOC)]

    # block (lt, jt) of Srel is identically zero unless piece A
    # (j <= 2l-1023) or piece B (l+2 <= j <= 2l+3) intersects it.
    def srel_block_nonzero(lt, jt):
        l1 = 128 * lt + 127
        j0, j1 = 128 * jt, 128 * jt + 127
        a = 2 * l1 - 1023 >= j0
        b = (j1 >= 128 * lt + 2) and (j0 <= 2 * l1 + 3)
        return a or b

    with tile.TileContext(nc) as tc:
        from contextlib import ExitStack

        with ExitStack() as outer:
            # ---------------- persistent pools ----------------
            persist = outer.enter_context(tc.tile_pool(name="persist", bufs=1))
            # projection outputs (live through whole kernel)
            qhT = persist.tile([128, NPAIR, L], bf16)  # [part, pair, l]
            khT = persist.tile([128, NPAIR, L], bf16)
            # vh with ones column per head: [part(j%128), jt, head, 65]
            vh = persist.tile([128, NLT, H_LOC, HD + 1], bf16)
            e2_sb = persist.tile([128, L], bf16)
            slab_sb = persist.tile([128, 640], bf16)
            ctxp = persist.tile([128, NPAIR, L], bf16)  # packed ctx^T per pair
            # all-ones stationary for the K=1 1/Z PE broadcast
            ones1 = persist.tile([1, 64], bf16, name="ones1")
            # persistent stripe staging tiles, pre-zeroed once; data spans are
            # rewritten per head, zero-col/tail spans stay zero across reuse
            stripes = [
                [persist.tile([128, 4, PAD], bf16, name=f"st{lh}{par}")
                 for par in range(2)]
                for lh in range(2)
            ]

            nc.sync.dma_start(out=e2_sb, in_=e2_d[:, :])
            nc.sync.dma_start(out=slab_sb, in_=slab_d[:, :])
            nc.vector.memset(vh[:, :, :, HD : HD + 1], 1.0)
            nc.vector.memset(ones1, 1.0)
            for lh in range(2):
                for par in range(2):
                    nc.vector.memset(stripes[lh][par], 0.0)

            # ---------------- phase 1+2: loads + projections ----
            with ExitStack() as outer2:
                sc_ps = outer2.enter_context(
                    tc.tile_pool(name="sc_ps", bufs=5, space="PSUM")
                )
                qe_ps = sc_ps  # QE shares the scores PSUM slots (tag "sc")
                ctx_ps = None  # opened after mm_ps closes (PSUM bank budget)
                attT = outer2.enter_context(tc.tile_pool(name="attT", bufs=4))
                srl = outer2.enter_context(tc.tile_pool(name="srl", bufs=2))
                zp = outer2.enter_context(tc.tile_pool(name="zp", bufs=2))

                ident = persist.tile([128, 128], bf16, name="ident")
                from concourse.masks import make_identity

                make_identity(nc, ident)

                # short-lived input pools opened last (LIFO close order)
                tin_blk = ExitStack()
                tin = tin_blk.enter_context(tc.tile_pool(name="tin", bufs=1))
                mm_ps = tin_blk.enter_context(
                    tc.tile_pool(name="mm_ps", bufs=2, space="PSUM")
                )

                qT = tin.tile([128, NDT, L], bf16, name="qT")
                kT = tin.tile([128, NDT, L], bf16, name="kT")
                vT = tin.tile([128, NDT, L], bf16, name="vT")
                wq_sb = tin.tile([128, NDT, DG], bf16, name="wq")
                wk_sb = tin.tile([128, NDT, DG], bf16, name="wk")
                wv_sb = tin.tile([128, NDT, DG], bf16, name="wv")

                def load_xT(dst, src_d):
                    src = bass.AP(
                        tensor=src_d,
                        offset=0,
                        ap=[[L, 128], [128 * L, NDT], [1, L]],
                    )
                    nc.sync.dma_start(out=dst, in_=src)

                def load_w(dst, src_d):
                    src = bass.AP(
                        tensor=src_d,
                        offset=0,
                        ap=[[DG, 128], [128 * DG, NDT], [1, DG]],
                    )
                    nc.sync.dma_start(out=dst, in_=src)

                # q + Wq first so qh projections (and QE) can start early;
                # whole-tensor batched loads (one trigger per tensor)
                load_w(wq_sb, wq_d)
                load_xT(qT, qT_d)
                load_w(wk_sb, wk_d)
                load_xT(kT, kT_d)
                load_w(wv_sb, wv_d)
                load_xT(vT, vT_d)

                def proj_pair(w_sb, xT, dst, p):
                    for lh in range(2):
                        ps = mm_ps.tile([128, 512], f32, name="proj_ps", tag="mm")
                        lsl = slice(512 * lh, 512 * (lh + 1))
                        for dt in range(NDT):
                            nc.tensor.matmul(
                                ps,
                                w_sb[:, dt, 128 * p : 128 * (p + 1)],
                                xT[:, dt, lsl],
                                start=(dt == 0),
                                stop=(dt == NDT - 1),
                            )
                        nc.scalar.copy(dst[:, p, lsl], ps)

                def vh_tile(jt):
                    ps = mm_ps.tile([128, 512], f32, name="vh_ps", tag="mm")
                    jsl = slice(128 * jt, 128 * (jt + 1))
                    for dt in range(NDT):
                        nc.tensor.matmul(
                            ps,
                            vT[:, dt, jsl],
                            wv_sb[:, dt, :],
                            start=(dt == 0),
                            stop=(dt == NDT - 1),
                        )
                    # scatter 512 d' columns into per-head [64] slots with one
                    # strided copy (dst strides over the 65-wide head slots)
                    pv = ps[0:128, :]
                    ps3 = bass.AP(
                        tensor=pv.tensor,
                        offset=pv.offset,
                        ap=[list(pv.ap)[0], [HD, H_LOC], [1, HD]],
                    )
                    nc.scalar.copy(vh[:, jt, :, 0:HD], ps3)

                def qe_lt(h, lt):
                    """QE + masked padded stripe row-block lt for head h;
                    after the 4th block of an l-half, emit the stripe DMA."""
                    p, hl = divmod(h, 2)
                    rows = slice(64 * hl, 64 * (hl + 1))
                    tp = (64 * hl, 0)
                    lh, a = divmod(lt, 4)
                    big = stripes[lh][h % 2]
                    l0 = 128 * lt
                    lsl = slice(l0, l0 + 128)
                    stripe = big[:, a, :]
                    # QE only over the m-range that survives the tri mask
                    if lt <= 3:
                        n0 = l0 + 128
                        psm = qe_ps.tile([128, 512], f32, name="qe", tag="sc")
                        nc.tensor.matmul(
                            psm[:, 0:n0],
                            qhT[rows, p, lsl],
                            e2_sb[rows, 0:n0],
                            start=True,
                            stop=True,
                            tile_position=tp,
                        )
                        nc.vector.tensor_tensor(
                            stripe[:, 1 : 1 + n0],
                            psm[:, 0:n0],
                            slab_sb[:, 512 - l0 : 640],
                            mult,
                        )
                    else:
                        psm = qe_ps.tile([128, 512], f32, name="qe", tag="sc")
                        nc.tensor.matmul(
                            psm,
                            qhT[rows, p, lsl],
                            e2_sb[rows, 0:512],
                            start=True,
                            stop=True,
                            tile_position=tp,
                        )
                        n1 = l0 + 128 - 512
                        psm2 = qe_ps.tile([128, 512], f32, name="qe", tag="sc")
                        nc.tensor.matmul(
                            psm2[:, 0:n1],
                            qhT[rows, p, lsl],
                            e2_sb[rows, 512 : 512 + n1],
                            start=True,
                            stop=True,
                            tile_position=tp,
                        )
                        if lt == 4:
                            nc.vector.tensor_tensor(
                                stripe[:, 1:513],
                                psm,
                                slab_sb[:, 0:512],
                                mult,
                            )
                        else:
                            # m < 512 is fully below the diagonal: copy
                            nc.vector.tensor_copy(stripe[:, 1:513], psm)
                        nc.vector.tensor_tensor(
                            stripe[:, 513 : 1 + l0 + 128],
                            psm2[:, 0:n1],
                            slab_sb[:, 1024 - l0 : 640],
                            mult,
                        )
                    if a == 3:
                        # one DMA for the 4 padded stripes of this l-half
                        dst = bass.AP(
                            tensor=scratch[h],
                            offset=512 * lh * PAD,
                            ap=[[PAD, 128], [128 * PAD, 4], [1, PAD]],
                        )
                        nc.sync.dma_start(out=dst, in_=big)

                def srel_load(h, lh):
                    if lh == 0:
                        # low l-half: sliding 640-wide j-window per lt
                        # (window start 128*lt covers all nonzero blocks)
                        srel = srl.tile([128, 4, 640], bf16, name="srel")
                        src = bass.AP(
                            tensor=scratch[h],
                            offset=L,
                            ap=[[L, 128], [128 * L + 128, 4], [1, 640]],
                        )
                    else:
                        # high l-half: dense
                        srel = srl.tile([128, 4, L], bf16, name="srel")
                        src = bass.AP(
                            tensor=scratch[h],
                            offset=(512 * lh + 1) * L,
                            ap=[[L, 128], [128 * L, 4], [1, L]],
                        )
                    nc.sync.dma_start(out=srel, in_=src)
                    return srel

                def scores_tile(h, lh, jt, srel, att):
                    """scores^T + Srel^T + exp for one (l-half, j-tile)."""
                    p, hl = divmod(h, 2)
                    rows = slice(64 * hl, 64 * (hl + 1))
                    tp = (64 * hl, 0)
                    lsl = slice(512 * lh, 512 * (lh + 1))
                    jsl = slice(128 * jt, 128 * (jt + 1))
                    ps = sc_ps.tile([128, 512], f32, name="sc", tag="sc")
                    nzs = [
                        a for a in range(4)
                        if srel_block_nonzero(4 * lh + a, jt)
                    ]
                    # scores^T = kh qh^T for this (j-tile, l-half)
                    nc.tensor.matmul(
                        ps,
                        khT[rows, p, jsl],
                        qhT[rows, p, lsl],
                        start=True,
                        stop=(len(nzs) == 0),
                        tile_position=tp,
                    )
                    # += Srel^T via PE transpose-by-identity
                    for i, a in enumerate(nzs):
                        if lh == 0:
                            jr = slice(128 * (jt - a), 128 * (jt - a) + 128)
                            chunk = srel[:, a, jr]
                        else:
                            chunk = srel[:, a, jsl]
                        nc.tensor.matmul(
                            ps[:, 128 * a : 128 * a + 128],
                            chunk,
                            ident,
                            start=False,
                            stop=(i == len(nzs) - 1),
                        )
                    nc.scalar.activation(att[:, jt, :], ps, Exp, scale=0.125)

                def attnv_part(h, halves, lh, jt, cps):
                    nc.tensor.matmul(
                        cps[0 : HD + 1, :],
                        vh[:, jt, h, :],
                        halves[lh][:, jt, :],
                        start=(jt == 0),
                        stop=(jt == NLT - 1),
                    )

                def attnv_finish(h, lh, cps):
                    p, hl = divmod(h, 2)
                    rows = slice(64 * hl, 64 * (hl + 1))
                    # 1/Z = exp(-ln Z) via two ScalarE activations (the
                    # Reciprocal table is blocked; DVE reciprocal is 3.3us)
                    zln = zp.tile([1, 512], f32, name="zln")
                    nc.scalar.activation(zln, cps[HD : HD + 1, :], Ln)
                    zinvb = zp.tile([1, 512], bf16, name="zinvb")
                    nc.scalar.activation(zinvb, zln, Exp, scale=-1.0)
                    # broadcast across 64 partitions with a K=1 matmul
                    zb = sc_ps.tile([64, 512], f32, name="zb", tag="sc")
                    nc.tensor.matmul(zb, ones1, zinvb, start=True, stop=True)
                    zbs = zp.tile([64, 512], bf16, name="zbs")
                    nc.scalar.copy(zbs, zb)
                    # normalize + pack into head-pair ctx^T (bf16)
                    nc.vector.tensor_tensor(
                        ctxp[rows, p, 512 * lh : 512 * (lh + 1)],
                        cps[0:HD, :],
                        zbs,
                        mult,
                    )

                # ---- emission: projections first ----
                for p in range(NPAIR):
                    proj_pair(wq_sb, qT, qhT, p)
                for lt in range(8):
                    qe_lt(0, lt)
                for lt in range(8):
                    qe_lt(1, lt)
                for p in range(NPAIR):
                    proj_pair(wk_sb, kT, khT, p)
                for jt in range(NLT):
                    vh_tile(jt)
                tin_blk.close()
                ctx_ps = outer2.enter_context(
                    tc.tile_pool(name="ctx_ps", bufs=3, space="PSUM")
                )
                # wo lives in the space vacated by the input tiles; loaded
                # here (well before the output projection)
                wop = outer2.enter_context(tc.tile_pool(name="wop", bufs=1))
                wo_sb = wop.tile([128, NPAIR, D], bf16, name="wo")
                wo_src = bass.AP(
                    tensor=wo_d, offset=0, ap=[[D, 128], [128 * D, NPAIR], [1, D]]
                )
                nc.sync.dma_start(out=wo_sb, in_=wo_src)
                ost = outer2.enter_context(tc.tile_pool(name="ost", bufs=4))

                # ---- main pipeline: scores(h) / attnV(h-1) / QE(h+2)
                # interleaved at j-tile granularity so the in-order PE
                # always has a ready instruction ----
                pend = None
                for h in range(H_LOC):
                    srel0 = srel_load(h, 0)
                    att0 = attT.tile([128, NLT, 512], bf16, name="attnT")
                    att1 = attT.tile([128, NLT, 512], bf16, name="attnT")
                    cps_prev = {}
                    for jt in range(NLT):
                        scores_tile(h, 0, jt, srel0, att0)
                        if pend is not None:
                            hp, halves = pend
                            if jt == 0:
                                cps_prev[0] = ctx_ps.tile(
                                    [128, 512], f32, name="cps", tag="cps"
                                )
                            attnv_part(hp, halves, 0, jt, cps_prev[0])
                        if jt % 2 == 1 and h + 2 < H_LOC:
                            qe_lt(h + 2, jt // 2)
                    if pend is not None:
                        attnv_finish(hp, 0, cps_prev[0])
                    srel1 = srel_load(h, 1)
                    for jt in range(NLT):
                        scores_tile(h, 1, jt, srel1, att1)
                        if pend is not None:
                            if jt == 0:
                                cps_prev[1] = ctx_ps.tile(
                                    [128, 512], f32, name="cps", tag="cps"
                                )
                            attnv_part(hp, halves, 1, jt, cps_prev[1])
                        if jt % 2 == 1 and h + 2 < H_LOC:
                            qe_lt(h + 2, 4 + jt // 2)
                    if pend is not None:
                        attnv_finish(hp, 1, cps_prev[1])
                    pend = (h, [att0, att1])

                # ---- tail: attnV of the last head, interleaved with the
                # first half of the output projection ----
                def outproj_unit(lt, jh, o):
                    lsl = slice(128 * lt, 128 * (lt + 1))
                    jsl = slice(512 * jh, 512 * (jh + 1))
                    ps = sc_ps.tile([128, 512], f32, name="op", tag="sc")
                    for p in range(NPAIR):
                        nc.tensor.matmul(
                            ps,
                            ctxp[:, p, lsl],
                            wo_sb[:, p, jsl],
                            start=(p == 0),
                            stop=(p == NPAIR - 1),
                        )
                    nc.scalar.copy(o[:, jsl], ps)

                hp, halves = pend
                o_tiles = {}
                cps0 = ctx_ps.tile([128, 512], f32, name="cps", tag="cps")
                for jt in range(NLT):
                    attnv_part(hp, halves, 0, jt, cps0)
                attnv_finish(hp, 0, cps0)
                cps1 = ctx_ps.tile([128, 512], f32, name="cps", tag="cps")
                for jt in range(NLT):
                    attnv_part(hp, halves, 1, jt, cps1)
                    # out-proj over the lh=0 l-tiles (ctxp cols 0..511 are
                    # complete once every head's lh=0 normalize is done)
                    lt, jh = jt // 2, jt % 2
                    if jh == 0:
                        o_tiles[lt] = ost.tile([128, D], bf16, name="o")
                    outproj_unit(lt, jh, o_tiles[lt])
                    if jh == 1:
                        lsl = slice(128 * lt, 128 * (lt + 1))
                        nc.sync.dma_start(out=out_d[lsl, :], in_=o_tiles[lt])
                attnv_finish(hp, 1, cps1)
                for lt in range(4, NLT):
                    o = ost.tile([128, D], bf16, name="o")
                    for jh in range(2):
                        outproj_unit(lt, jh, o)
                    lsl = slice(128 * lt, 128 * (lt + 1))
                    nc.sync.dma_start(out=out_d[lsl, :], in_=o)

    nc.compile()
    return nc


TRACE = False
TRACE_KWARGS = {}
LAST_RESULT = None

_NC_CACHE = None


def _get_nc():
    global _NC_CACHE
    if _NC_CACHE is None:
        _NC_CACHE = _build_bass()
    return _NC_CACHE


def make_in_maps(k, v, q, E, Wk, Wv, Wq, Wo):
    """Host-side sharding: returns per-core input dicts."""
    eT = np.ascontiguousarray(E[MAX_SEQ - L :, :].T)  # [64, 1024]
    e2 = np.concatenate([eT, eT], axis=0).astype(BF16)  # [128, 1024]
    slab = (
        (np.arange(640)[None, :] - 512) <= np.arange(128)[:, None]
    ).astype(BF16)
    qkvT = {}
    for b in range(B):
        qkvT[b] = (
            np.ascontiguousarray(np.asarray(q[b]).T).astype(BF16),
            np.ascontiguousarray(np.asarray(k[b]).T).astype(BF16),
            np.ascontiguousarray(np.asarray(v[b]).T).astype(BF16),
        )
    in_maps = []
    for core in range(NCORES):
        b, hg = divmod(core, 2)
        csl = slice(DG * hg, DG * (hg + 1))
        qTb, kTb, vTb = qkvT[b]
        in_maps.append(
            {
                "qT": qTb,
                "kT": kTb,
                "vT": vTb,
                "wq": np.ascontiguousarray(Wq[:, csl]).astype(BF16),
                "wk": np.ascontiguousarray(Wk[:, csl]).astype(BF16),
                "wv": np.ascontiguousarray(Wv[:, csl]).astype(BF16),
                "wo": np.ascontiguousarray(Wo[DG * hg : DG * (hg + 1), :]).astype(BF16),
                "e2": e2,
                "slab": slab,
            }
        )
    return in_maps


def kernel(
    k,
    v,
    q,
    mask,
    E,
    Wk,
    bk,
    Wv,
    bv,
    Wq,
    bq,
    Wo,
    bo,
):
    k = np.asarray(k, np.float32)
    v = np.asarray(v, np.float32)
    q = np.asarray(q, np.float32)
    E = np.asarray(E, np.float32)
    Wk = np.asarray(Wk, np.float32)
    Wv = np.asarray(Wv, np.float32)
    Wq = np.asarray(Wq, np.float32)
    Wo = np.asarray(Wo, np.float32)
    mask = np.asarray(mask)
    assert bool(mask.all()), "kernel specialized for all-true mask"
    for bias in (bk, bv, bq):
        assert not np.any(np.asarray(bias)), "kernel specialized for zero qkv biases"
    bo = np.asarray(bo, np.float32)

    from concourse.bass_utils import run_bass_kernel_spmd

    nc = _get_nc()
    in_maps = make_in_maps(k, v, q, E, Wk, Wv, Wq, Wo)
    res = run_bass_kernel_spmd(
        nc, in_maps, core_ids=list(range(NCORES)), trace=TRACE, **TRACE_KWARGS
    )
    global LAST_RESULT
    LAST_RESULT = res
    out = np.zeros((B, L, D), np.float32)
    for core in range(NCORES):
        b = core // 2
        out[b] += np.asarray(res.results[core]["out"], np.float32)
    out += bo[None, None, :]
    return out
